# revision 1
# baseline (speedup 1.0000x reference)
"""Trainium2 Bass kernel for AttentionDownsampler (nn_AttentionDownsampler_10264971837445).

Math (per batch b):
  patches[b, Y, X, p=(y,xi), c] = hr[b, c, 14Y+y, 14X+xi]
  logits[b, Y, X, p] = sum_c patches * w[c] + ab
  l2 = logits * mask[b, Y, X] * wkk[p] + bkk[p]
  attn = softmax_p(l2)
  out[b, c, Y, X] = sum_p patches[..., p, c] * attn[p]

Sharding: 8 cores = 4 batches x 2 halves of the H(=Y) axis. Per-core shard is
patch-contiguous on the host: [384, 8 rows, 16 X, 196 px], sent as bf16
(weighted-average output keeps ~4e-3 rel err, well under the 2e-2 gate).

Per-core kernel (4 row-PAIR iterations):
  - DMA 3 c-chunk tiles [128, 2 rows, 16 X, 196 px] (bf16)
  - PE scoring in bf16 (1 cyc/col): 48 one-hot matmuls with rhs spanning the
    row pair (N=392) -> PSUM lg2 [16, 392]; row m holds logits[X=m] for
    row A in cols 0:196 and row B in cols 196:392.
  - softmax without max-subtraction (|l2| <= ~6 so exp is fp32-safe):
    t2 = lg2 * (mask*wkk) [DVE], ex = Exp(t2) [ACT], then per row
    attn_un = ex_row * E with esum accumulated [DVE AMR], E =
    exp(ab*mask*wkk + bkk) host-precomputed; one reciprocal per pair;
    attn = attn_un * rcp -> bf16 [DVE tensor_scalar].
  - attn broadcast: DRAM bounce in bf16 (SBUF->DRAM 6KB, then a
    partition-replicated DRAM->SBUF read -> attnB [128, 16, 196]).
    SBUF->SBUF was 8x slower (single-queue serialization); the bounce
    spreads across DMA engines.
  - pass B, software-pipelined one row deep (pass B of row r-1 is emitted
    after row r's broadcast so DVE/GpSimd always have ready work): per
    (row, chunk) unit, prod = data*attn as bf16 [12 units on GpSimd
    tensor_mul, 12 on DVE tensor_mul], then sum over p via DVE 3D
    tensor_reduce (axis=X) except 4 mid-schedule units on ACT
    (activation Copy + accum_out per X). GpSimd mults are emitted before
    DVE mults and own-product reduces before GpSimd-dependent ones, so
    DVE's in-order queue never head-of-line blocks on GpSimd.
  - outputs accumulate in SBUF [128, 8, 16] per chunk; one DMA per chunk.

Engine budget per core (measured): DVE ~110us (24 reduces at 3.4us
dominate; no faster segmented-reduce exists off-PE), GpSimd ~90us,
PE ~43us, ACT ~60us. The chip throttles (~52% util cap) when several
engines run hot, so span tracks total engine work; GpSimd/ACT shares
are kept moderate. Scoring matmuls are emitted k-outer and softmax
constants load after the first data tiles, so PE/DVE start early.
HW exec ~187us (reproducible +-0.5us), vs 247us for the fp32 AMR-based
baseline.
"""

import sys

for _p in ("/opt/trn_rl_repo", "/root/.axon_site/_ro/trn_rl_repo"):
    if _p not in sys.path:
        sys.path.append(_p)

import ml_dtypes
import numpy as np

import concourse.bacc as bacc
import concourse.bass as bass_mod
import concourse.mybir as mybir
import concourse.tile as tile
from concourse.bass_utils import run_bass_kernel_spmd

K = 14          # patch size
C = 384         # channels
CCH = 128       # channel chunk (partitions)
NCH = C // CCH  # 3 chunks
NX = 16         # patches across W
P = K * K       # 196 pixels per patch
W2 = 2 * P      # 392 columns: a row-pair in one scoring matmul group
NCORES = 8
NROW = 8
NPAIRS = NROW // 2

FP32 = mybir.dt.float32
BF16 = mybir.dt.bfloat16

# pass B: each of the 24 (row, chunk) units does mult (DVE tensor_mul at
# 2x bf16, or GpSimd tensor_mul) into a bf16 prod tile, then a 3D DVE
# tensor_reduce (axis=X) or 16 ACT activation+accum ops as the reduce.


def build_nc(nrow=NROW):
    """Build the SPMD Bass program (identical on all cores)."""
    nc = bacc.Bacc("TRN2", target_bir_lowering=False, debug=False,
                   num_devices=NCORES)

    # patch-grouped shard: [c, row, X, p]
    hr = nc.dram_tensor("hr", [C, nrow, NX, P], BF16, kind="ExternalInput")
    # one-hot scorer weights: woh[c, X, m] = w[c] if m == X else 0
    woh = nc.dram_tensor("woh", [C, NX, NX], BF16, kind="ExternalInput")
    # mw2[m, pair, ri*196+p] = mask[2*pair+ri, m] * wkk[p]
    mw2 = nc.dram_tensor("mw2", [NX, NPAIRS, W2], FP32, kind="ExternalInput")
    # e196[m, r, p] = exp(ab * mask[r, m] * wkk[p] + bkk[p])
    e196 = nc.dram_tensor("e196", [NX, nrow, P], FP32, kind="ExternalInput")
    out = nc.dram_tensor("out", [C, nrow, NX], FP32, kind="ExternalOutput")
    attn_dram = nc.dram_tensor("attn_scratch", [nrow, NX, P], BF16)

    with tile.TileContext(nc) as tc:
        _emit(tc, nc, nrow, hr, woh, mw2, e196, out, attn_dram)
    nc.finalize()
    return nc


def _emit(tc, nc, nrow, hr, woh, mw2, e196, out, attn_dram):
    import contextlib

    # per-unit engine assignment (u = 3*row + chunk, 24 units). The chip
    # throttles to ~52% util when several engines run hot, so span tracks
    # total engine work; GpSimd mults (7.5us vs DVE 1.7-3.3) and ACT
    # reduces (14.3 vs 3.4) inflate it, but taking ~half the mults on
    # GpSimd and 4 mid-schedule reduces on ACT keeps the serial DVE chain
    # short without blowing the power budget (measured optimum).
    gps_mult = [u % 2 == 0 for u in range(24)]
    act_red = [u in (6, 10, 14, 16) for u in range(24)]

    ctx = contextlib.ExitStack()
    with ctx:
        singles = ctx.enter_context(tc.tile_pool(name="singles", bufs=1))
        data_pool = ctx.enter_context(tc.tile_pool(name="data", bufs=9))
        small = ctx.enter_context(tc.tile_pool(name="small", bufs=3))
        attnb_pool = ctx.enter_context(tc.tile_pool(name="attnb", bufs=3))
        prod_pool = ctx.enter_context(tc.tile_pool(name="prod", bufs=6))
        scratch_pool = ctx.enter_context(tc.tile_pool(name="scratch", bufs=1))
        psum_lg = ctx.enter_context(
            tc.tile_pool(name="psum_lg", bufs=2, space="PSUM"))

        # ---- constants (loaded once) ----
        woh_sb = singles.tile([CCH, NCH, NX, NX], BF16)
        for k in range(NCH):
            nc.sync.dma_start(out=woh_sb[:, k, :, :],
                              in_=woh[k * CCH:(k + 1) * CCH, :, :])
        mw2_sb = singles.tile([NX, NPAIRS, W2], FP32)
        e196_sb = singles.tile([NX, nrow, P], FP32)

        scratch2 = scratch_pool.tile([CCH, P], BF16, tag="scratch2")
        osb = [singles.tile([CCH, nrow, NX], FP32, name=f"osb{k}",
                            tag=f"osb{k}") for k in range(NCH)]

        def pass_b(r, dkp, ri, attnB):
            # Emit all GpSimd mults first, then DVE mults, then reduces
            # (own-product reduces before GpSimd-dependent ones) so DVE's
            # in-order queue never blocks its own mults behind a reduce
            # that waits on GpSimd.
            prods = {}
            order = sorted(range(NCH), key=lambda k: not gps_mult[3 * r + k])
            for k in order:
                prod = prod_pool.tile([CCH, NX, P], BF16, tag="prod")
                prods[k] = prod
                if gps_mult[3 * r + k]:
                    nc.gpsimd.tensor_mul(prod, dkp[k][:, ri, :, :], attnB)
                else:
                    nc.vector.tensor_mul(prod, dkp[k][:, ri, :, :], attnB)
            for k in sorted(range(NCH),
                            key=lambda k: bool(gps_mult[3 * r + k])):
                u = 3 * r + k
                if act_red[u]:
                    for X in range(NX):
                        nc.scalar.activation(
                            scratch2, prods[k][:, X, :],
                            mybir.ActivationFunctionType.Copy,
                            accum_out=osb[k][:, r, X:X + 1])
                else:
                    nc.vector.tensor_reduce(
                        osb[k][:, r, :], prods[k], axis=mybir.AxisListType.X,
                        op=mybir.AluOpType.add)

        pending = []
        consts_loaded = False
        for pr in range(NPAIRS):
            # ---- load data tiles (one row pair) ----
            dk = []
            for k in range(NCH):
                t = data_pool.tile([CCH, 2, NX, P], BF16, tag="data")
                nc.sync.dma_start(
                    out=t, in_=hr[k * CCH:(k + 1) * CCH, 2 * pr:2 * pr + 2,
                                  :, :])
                dk.append(t)
            if not consts_loaded:
                # softmax constants are first needed ~20us in; keep them off
                # the sync queue until the first data tiles are streaming
                nc.sync.dma_start(out=mw2_sb, in_=mw2[:, :, :])
                nc.sync.dma_start(out=e196_sb, in_=e196[:, :, :])
                consts_loaded = True

            # ---- scoring: 48 one-hot matmuls (N=392), one accum group,
            # k-outer so PE starts as soon as chunk 0's DMA lands ----
            lg2 = psum_lg.tile([NX, W2], FP32, tag="lg")
            for k in range(NCH):
                for X in range(NX):
                    nc.tensor.matmul(
                        lg2[:, :],
                        woh_sb[:, k, X, :],
                        dk[k][:, :, X, :],
                        start=(k == 0 and X == 0),
                        stop=(k == NCH - 1 and X == NX - 1),
                    )

            # ---- softmax over p (no max subtraction; |l2| small) ----
            t2 = small.tile([NX, W2], FP32, tag="t2")
            nc.vector.tensor_mul(t2, lg2[:, :], mw2_sb[:, pr, :])
            ex = small.tile([NX, W2], FP32, tag="ex")
            nc.scalar.activation(ex, t2, mybir.ActivationFunctionType.Exp)
            attn_un = small.tile([NX, 2, P], FP32, tag="attn_un")
            esum = small.tile([NX, 2], FP32, tag="esum")
            for ri in range(2):
                nc.vector.affine_mul_reduce(
                    out=attn_un[:, ri, :], accum_out=esum[:, ri:ri + 1],
                    in0=ex[:, ri * P:(ri + 1) * P],
                    in1=e196_sb[:, 2 * pr + ri, :],
                    scale=1.0, bias=0.0)
            rcp = small.tile([NX, 2], FP32, tag="rcp")
            nc.vector.reciprocal(rcp, esum)
            attn = small.tile([NX, 2, P], BF16, tag="attn")
            for ri in range(2):
                nc.vector.tensor_scalar_mul(attn[:, ri, :], attn_un[:, ri, :],
                                            rcp[:, ri:ri + 1])

            # ---- per row: broadcast, then pass B of the previous row ----
            for ri in range(2):
                r = 2 * pr + ri
                attnB = attnb_pool.tile([CCH, NX, P], BF16, tag="attnB")
                nc.gpsimd.dma_start(out=attn_dram[r, :, :], in_=attn[:, ri, :])
                _src = attn_dram[r, :, :]
                _bsrc = bass_mod.AP(tensor=_src.tensor, offset=_src.offset,
                                    ap=[[0, CCH], *_src.ap])
                nc.gpsimd.dma_start(out=attnB, in_=_bsrc)
                pending.append((r, dk, ri, attnB))
                if len(pending) > 1:
                    pass_b(*pending.pop(0))

        for args in pending:
            pass_b(*args)

        for k in range(NCH):
            nc.sync.dma_start(out=out[k * CCH:(k + 1) * CCH, :, :],
                              in_=osb[k])


_NC_CACHE = {}


def _get_nc(nrow=NROW):
    if nrow not in _NC_CACHE:
        _NC_CACHE[nrow] = build_nc(nrow)
    return _NC_CACHE[nrow]


def regroup_shard(hr_slice):
    """[384, 112, 224] -> patch-grouped bf16 [384, 8, 16, 196]."""
    c, h, w = hr_slice.shape
    g = hr_slice.reshape(c, h // K, K, w // K, K).transpose(0, 1, 3, 2, 4)
    return np.ascontiguousarray(
        g.reshape(c, h // K, w // K, P)).astype(ml_dtypes.bfloat16)


def make_in_maps(hr_feats, guidance, attn_w, attn_b, w_kk, b_kk, dropout_mask,
                 nrow=NROW):
    b = hr_feats.shape[0]
    w = np.asarray(attn_w, np.float32)[0]                      # [384]
    ab = np.float32(np.asarray(attn_b)[0])
    wkk_flat = np.asarray(w_kk, np.float32).reshape(-1)        # [196]
    bkk_flat = np.asarray(b_kk, np.float32).reshape(-1)        # [196]
    mask = np.asarray(dropout_mask).astype(np.float32)[..., 0]  # [b, H, W]

    woh = np.zeros((C, NX, NX), np.float32)
    woh[:, np.arange(NX), np.arange(NX)] = w[:, None]          # [c, X, m]
    woh = woh.astype(ml_dtypes.bfloat16)

    in_maps = []
    for core in range(NCORES):
        bi, half = divmod(core, 2)
        bi = bi % b
        hrg = regroup_shard(
            np.asarray(hr_feats[bi, :, 112 * half:112 * half + K * nrow, :],
                       np.float32))
        mrow = mask[bi, 8 * half:8 * half + nrow, :]           # [nrow, 16]
        mcol = np.ascontiguousarray(mrow.T)                    # [16(X), nrow]
        # mw2[m, pair, ri*196+p] = mask[2*pair+ri, m] * wkk[p]
        mw2 = (mcol[:, :, None] * wkk_flat[None, None, :])     # [16, nrow, 196]
        mw2 = np.ascontiguousarray(
            mw2.reshape(NX, NPAIRS, W2)).astype(np.float32)
        e196 = np.ascontiguousarray(
            np.exp(ab * mcol[:, :, None] * wkk_flat[None, None, :]
                   + bkk_flat[None, None, :])).astype(np.float32)
        in_maps.append({
            "hr": hrg, "woh": woh, "mw2": mw2, "e196": e196,
        })
    return in_maps


def kernel(hr_feats, guidance, attn_w, attn_b, w_kk, b_kk, dropout_mask,
           trace=False):
    hr_feats = np.asarray(hr_feats, np.float32)
    b, c, h, wimg = hr_feats.shape
    H = h // K
    nc = _get_nc(NROW)
    in_maps = make_in_maps(hr_feats, guidance, attn_w, attn_b, w_kk, b_kk,
                           dropout_mask)
    res = run_bass_kernel_spmd(nc, in_maps, core_ids=list(range(NCORES)),
                               trace=trace)
    full = np.empty((b, C, H, NX), np.float32)
    for core in range(NCORES):
        bi, half = divmod(core, 2)
        full[bi, :, 8 * half:8 * half + 8, :] = res.results[core]["out"]
    if trace:
        return full, res
    return full



# revision 2
# speedup vs baseline: 1.1760x; 1.1760x over previous
"""Trainium2 Bass kernel for AttentionDownsampler (nn_AttentionDownsampler_10264971837445).

Math (per batch b):
  patches[b, Y, X, p=(y,xi), c] = hr[b, c, 14Y+y, 14X+xi]
  logits[b, Y, X, p] = sum_c patches * w[c] + ab
  l2 = logits * mask[b, Y, X] * wkk[p] + bkk[p]
  attn = softmax_p(l2)
  out[b, c, Y, X] = sum_p patches[..., p, c] * attn[p]

Sharding: 8 cores = 4 batches x 2 halves of the H(=Y) axis; per-core shard is
8 rows x 16 X patches of 196 px x 384 c.

All bulk compute runs on the PE (2.4 GHz, 1 col/cyc bf16); DVE/ACT only do
small softmax tiles. The data is sent twice in bf16 (DVE has no fast reduce:
tensor_reduce/AMR run 1 col/cyc at 0.96 GHz, so any vector-engine reduction
of the 9.6M-element shard costs ~100us):

  - c-major copy [384c, row, X, 196p] feeds the scoring matmuls (contraction
    over c on partitions): 48 one-hot matmuls per row-pair -> PSUM
    lg2[16X, 392] as in the previous kernel.
  - p-major copy [98p, 2ph, row, X, 385] feeds the reduction (contraction
    over p on partitions). Column 384 is a constant 1.0 -> esum arrives in
    the same PSUM tile for free.

Softmax: t2 = lg2*mw2 + lkk2 (DVE; e196 factor folded in log space), ex =
Exp(t2) -> bf16 (ACT). Per row: PE-transpose ex[16, 98] -> [98, 16] per
p-half, DVE-copy the 16 columns onto the stride-17 diagonal of a zeroed
one-hot tile oh[98, 16X*16m] (off-diagonal stays zero forever; 2-deep parity
buffering), then 32 matmuls (16 X x 2 ph) with lhsT = oh[:, X, :] accumulate
psum[16X, 385]: row X only receives attn_X * D_X (other columns of oh are
zero), col 384 = esum. Normalize: reciprocal + tensor_scalar_mul -> SBUF,
one output DMA [16, 8, 384] fp32 at the end (host transposes back).

Budget (cost model): DMA 2x19.3MB = 107us @360GB/s (span driver), PE ~75us
(31 scoring + 41 reduction + transposes), DVE ~10us, ACT ~3us, GpSimd 0.
The previous kernel ran the reduction on DVE/GpSimd (~200us of engine work,
attn DRAM-bounce broadcast): 186-198us measured. This one targets ~115us.
"""

import sys

for _p in ("/opt/trn_rl_repo", "/root/.axon_site/_ro/trn_rl_repo"):
    if _p not in sys.path:
        sys.path.append(_p)

import ml_dtypes
import numpy as np

import concourse.bacc as bacc
import concourse.bass as bass_mod
import concourse.mybir as mybir
import concourse.tile as tile
from concourse.bass_utils import run_bass_kernel_spmd

K = 14          # patch size
C = 384         # channels
CCH = 128       # channel chunk (partitions)
NCH = C // CCH  # 3 chunks
NX = 16         # patches across W
P = K * K       # 196 pixels per patch
PH = P // 2     # 98: pixels per p-half (reduction contraction tile)
W2 = 2 * P      # 392 columns: a row-pair in one scoring matmul group
NC1 = C + 1     # 385: reduction rhs columns (384 c + ones column)
NCORES = 8
NROW = 8
NPAIRS = NROW // 2

FP32 = mybir.dt.float32
BF16 = mybir.dt.bfloat16


def build_nc(nrow=NROW):
    """Build the SPMD Bass program (identical on all cores)."""
    nc = bacc.Bacc("TRN2", target_bir_lowering=False, debug=False,
                   num_devices=NCORES)

    # c-major shard: [c, row, X, p]
    hr = nc.dram_tensor("hr", [C, nrow, NX, P], BF16, kind="ExternalInput")
    # p-major shard: [p%98, ph, row, X, 384c + ones]
    hrt = nc.dram_tensor("hrt", [PH, 2, nrow, NX, NC1], BF16,
                         kind="ExternalInput")
    # one-hot scorer weights: woh[c, X, m] = w[c] if m == X else 0
    woh = nc.dram_tensor("woh", [C, NX, NX], BF16, kind="ExternalInput")
    # mw2[m, pair, ri*196+p] = mask[2*pair+ri, m] * wkk[p]
    mw2 = nc.dram_tensor("mw2", [NX, NPAIRS, W2], FP32, kind="ExternalInput")
    # lkk2[m, pair, ri*196+p] = ab*mask[2*pair+ri, m]*wkk[p] + bkk[p]
    lkk2 = nc.dram_tensor("lkk2", [NX, NPAIRS, W2], FP32, kind="ExternalInput")
    # identity for PE transpose
    ident = nc.dram_tensor("ident", [NX, NX], BF16, kind="ExternalInput")
    # out_t[row, X, c] (host transposes back to [c, row, X])
    out = nc.dram_tensor("out", [NX, nrow, C], FP32, kind="ExternalOutput")

    with tile.TileContext(nc) as tc:
        _emit(tc, nc, nrow, hr, hrt, woh, mw2, lkk2, ident, out)
    nc.finalize()
    return nc


def _emit(tc, nc, nrow, hr, hrt, woh, mw2, lkk2, ident, out):
    import contextlib

    ctx = contextlib.ExitStack()
    with ctx:
        singles = ctx.enter_context(tc.tile_pool(name="singles", bufs=1))
        cdata = ctx.enter_context(tc.tile_pool(name="cdata", bufs=6))
        pdata = ctx.enter_context(tc.tile_pool(name="pdata", bufs=3))
        small = ctx.enter_context(tc.tile_pool(name="small", bufs=2))
        psum_lg = ctx.enter_context(
            tc.tile_pool(name="psum_lg", bufs=2, space="PSUM"))
        psum_r = ctx.enter_context(
            tc.tile_pool(name="psum_r", bufs=2, space="PSUM"))
        psum_t = ctx.enter_context(
            tc.tile_pool(name="psum_t", bufs=2, space="PSUM"))

        # ---- constants (loaded once, small) ----
        woh_sb = singles.tile([CCH, NCH, NX, NX], BF16)
        for k in range(NCH):
            nc.sync.dma_start(out=woh_sb[:, k, :, :],
                              in_=woh[k * CCH:(k + 1) * CCH, :, :])
        mw2_sb = singles.tile([NX, NPAIRS, W2], FP32)
        lkk2_sb = singles.tile([NX, NPAIRS, W2], FP32)
        ident_sb = singles.tile([NX, NX], BF16)
        nc.sync.dma_start(out=mw2_sb, in_=mw2[:, :, :])
        nc.sync.dma_start(out=lkk2_sb, in_=lkk2[:, :, :])
        nc.sync.dma_start(out=ident_sb, in_=ident[:, :])

        # one-hot attn tiles [98, 16X * 16m], diagonal (stride 17) rewritten
        # per row, zeros elsewhere written once. 2-deep parity buffering.
        oh = [[singles.tile([PH, NX * NX], BF16, name=f"oh{par}{ph}",
                            tag=f"oh{par}{ph}") for ph in range(2)]
              for par in range(2)]
        for par in range(2):
            for ph in range(2):
                nc.vector.memset(oh[par][ph], 0.0)

        # output accumulator [16X, row, 384c] fp32
        osb = singles.tile([NX, nrow, C], FP32, name="osb", tag="osb")

        for pr in range(NPAIRS):
            # ---- DMA: c-major pair tiles, then p-major row tiles ----
            dk = []
            for k in range(NCH):
                t = cdata.tile([CCH, 2, NX, P], BF16, tag="cdata")
                nc.sync.dma_start(
                    out=t, in_=hr[k * CCH:(k + 1) * CCH, 2 * pr:2 * pr + 2,
                                  :, :])
                dk.append(t)
            pt = []
            for ri in range(2):
                t = pdata.tile([PH, 2, NX, NC1], BF16, tag="pdata")
                nc.sync.dma_start(out=t, in_=hrt[:, :, 2 * pr + ri, :, :])
                pt.append(t)

            # ---- scoring: 48 one-hot matmuls (N=392), one accum group,
            # k-outer so PE starts as soon as chunk 0's DMA lands ----
            lg2 = psum_lg.tile([NX, W2], FP32, tag="lg")
            for k in range(NCH):
                for X in range(NX):
                    nc.tensor.matmul(
                        lg2[:, :],
                        woh_sb[:, k, X, :],
                        dk[k][:, :, X, :],
                        start=(k == 0 and X == 0),
                        stop=(k == NCH - 1 and X == NX - 1),
                    )

            # ---- softmax numerator: ex = exp(lg2*mw2 + lkk2) -> bf16 ----
            t2 = small.tile([NX, W2], FP32, tag="t2")
            nc.vector.tensor_mul(t2, lg2[:, :], mw2_sb[:, pr, :])
            nc.vector.tensor_add(t2, t2, lkk2_sb[:, pr, :])
            ex = small.tile([NX, W2], BF16, tag="ex")
            nc.scalar.activation(ex, t2, mybir.ActivationFunctionType.Exp)

            # ---- per row: transpose ex, scatter diagonal, reduce on PE ----
            for ri in range(2):
                r = 2 * pr + ri
                par = r % 2
                for ph in range(2):
                    tp = psum_t.tile([PH, NX], BF16, tag="tp")
                    nc.tensor.transpose(
                        tp[:, :], ex[:, ri * P + ph * PH:ri * P + (ph + 1) * PH],
                        ident_sb[:, :])
                    # scatter [98, 16] onto the stride-17 diagonal of oh
                    dst = oh[par][ph][:, :]
                    diag = bass_mod.AP(tensor=dst.tensor, offset=dst.offset,
                                       ap=[dst.ap[0], [NX + 1, NX]])
                    nc.vector.tensor_copy(diag, tp[:, :])
                pr_ps = psum_r.tile([NX, NC1], FP32, tag="pr")
                for X in range(NX):
                    for ph in range(2):
                        nc.tensor.matmul(
                            pr_ps[:, :],
                            oh[par][ph][:, X * NX:(X + 1) * NX],
                            pt[ri][:, ph, X, :],
                            start=(X == 0 and ph == 0),
                            stop=(X == NX - 1 and ph == 1),
                        )
                # normalize: out[X, c] = psum[X, c] / psum[X, 384]
                rcp = small.tile([NX, 1], FP32, tag="rcp")
                nc.vector.reciprocal(rcp, pr_ps[:, C:NC1])
                nc.vector.tensor_scalar_mul(osb[:, r, :], pr_ps[:, 0:C], rcp)

        nc.sync.dma_start(out=out[:, :, :], in_=osb)


_NC_CACHE = {}


def _get_nc(nrow=NROW):
    if nrow not in _NC_CACHE:
        _NC_CACHE[nrow] = build_nc(nrow)
    return _NC_CACHE[nrow]


def regroup_shard(hr_slice):
    """[384, 112, 224] -> patch-grouped fp32 [384, 8, 16, 196]."""
    c, h, w = hr_slice.shape
    g = hr_slice.reshape(c, h // K, K, w // K, K).transpose(0, 1, 3, 2, 4)
    return np.ascontiguousarray(g.reshape(c, h // K, w // K, P))


def make_in_maps(hr_feats, guidance, attn_w, attn_b, w_kk, b_kk, dropout_mask,
                 nrow=NROW):
    b = hr_feats.shape[0]
    w = np.asarray(attn_w, np.float32)[0]                      # [384]
    ab = np.float32(np.asarray(attn_b)[0])
    wkk_flat = np.asarray(w_kk, np.float32).reshape(-1)        # [196]
    bkk_flat = np.asarray(b_kk, np.float32).reshape(-1)        # [196]
    mask = np.asarray(dropout_mask).astype(np.float32)[..., 0]  # [b, H, W]

    woh = np.zeros((C, NX, NX), np.float32)
    woh[:, np.arange(NX), np.arange(NX)] = w[:, None]          # [c, X, m]
    woh = woh.astype(ml_dtypes.bfloat16)
    ident = np.eye(NX, dtype=ml_dtypes.bfloat16)

    in_maps = []
    for core in range(NCORES):
        bi, half = divmod(core, 2)
        bi = bi % b
        hrg = regroup_shard(
            np.asarray(hr_feats[bi, :, 112 * half:112 * half + K * nrow, :],
                       np.float32))                            # [384, 8, 16, 196] f32
        hrc = hrg.astype(ml_dtypes.bfloat16)                   # c-major copy
        # p-major copy [98, 2ph, row, X, 385] with ones column
        hp = hrg.transpose(3, 1, 2, 0).reshape(2, PH, nrow, NX, C)
        hp = hp.transpose(1, 0, 2, 3, 4)                       # [98, 2, 8, 16, 384]
        hrt = np.empty((PH, 2, nrow, NX, NC1), ml_dtypes.bfloat16)
        hrt[..., 0:C] = hp.astype(ml_dtypes.bfloat16)
        hrt[..., C] = np.float32(1.0)
        mrow = mask[bi, 8 * half:8 * half + nrow, :]           # [nrow, 16]
        mcol = np.ascontiguousarray(mrow.T)                    # [16(X), nrow]
        # mw2[m, pair, ri*196+p] = mask[2*pair+ri, m] * wkk[p]
        mw2 = (mcol[:, :, None] * wkk_flat[None, None, :])     # [16, nrow, 196]
        lkk2 = ab * mw2 + bkk_flat[None, None, :]
        mw2 = np.ascontiguousarray(
            mw2.reshape(NX, NPAIRS, W2)).astype(np.float32)
        lkk2 = np.ascontiguousarray(
            lkk2.reshape(NX, NPAIRS, W2)).astype(np.float32)
        in_maps.append({
            "hr": hrc, "hrt": hrt, "woh": woh, "mw2": mw2, "lkk2": lkk2,
            "ident": ident,
        })
    return in_maps


def kernel(hr_feats, guidance, attn_w, attn_b, w_kk, b_kk, dropout_mask,
           trace=False):
    hr_feats = np.asarray(hr_feats, np.float32)
    b, c, h, wimg = hr_feats.shape
    H = h // K
    nc = _get_nc(NROW)
    in_maps = make_in_maps(hr_feats, guidance, attn_w, attn_b, w_kk, b_kk,
                           dropout_mask)
    res = run_bass_kernel_spmd(nc, in_maps, core_ids=list(range(NCORES)),
                               trace=trace)
    full = np.empty((b, C, H, NX), np.float32)
    for core in range(NCORES):
        bi, half = divmod(core, 2)
        # out_t[X, row, c] -> [c, row, X]
        full[bi, :, 8 * half:8 * half + 8, :] = \
            res.results[core]["out"].transpose(2, 1, 0)
    if trace:
        return full, res
    return full
